# revision 16
# baseline (speedup 1.0000x reference)
"""Trainium2 Bass kernel for nn_CropModule: per-sample crop + bilinear resize.

Contract: kernel(img [128,3,480,480] f32, box [128,4] f32) -> [128, 150528] f32.

Strategy (pure data parallel, 16 samples per NeuronCore across 8 cores):
  * Host converts the image to bf16 HWC layout ([B,480,480,3]) and computes,
    per sample, the 240x240 crop window origin, sparse bilinear tables
    RyT/RxT [240,224] bf16 (2 nonzeros per output column), and one gather
    offset per (sample, partition).
  * Device, per sample: ONE gpsimd indirect DMA gathers the whole 3-channel
    window as 120 partition rows of 2160 contiguous elements (HWC window
    rows 2p / 2p+1 x 3 channels, plus a junk gap bridging the row stride).
    Per channel, two accumulating bf16 matmul passes resample y then x
    (channel deinterleave is a stride-3 access pattern on the stationary
    operand):
        mid[x, oy']  = sum_y  W[y, x] * RyT[y, oy']     (V pass)
        out[oy', ox] = sum_x mid[x, oy'] * RxT[x, ox]   (H pass)
    oy' columns are permuted even-first so the H pass yields even/odd row
    planes whose interleave is a contiguous [112, 448] DRAM row-pair write.
  * Output is written bf16 and upcast to f32 on host.
"""
from contextlib import ExitStack

import numpy as np
import ml_dtypes

import concourse.bass as bass
import concourse.mybir as mybir
import concourse.tile as tile
from concourse.bass_utils import run_bass_kernel_spmd
from concourse.vector_clock import ScopedClock

IMG = 480
OUT = 224
WIN = 240
RUN = 2160         # 2 HWC window-row spans (720 elems) bridged by 720 junk
BATCH = 128
N_CORES = 8
NSAMP = BATCH // N_CORES

F32 = mybir.dt.float32
BF16 = mybir.dt.bfloat16
I32 = mybir.dt.int32

BF = ml_dtypes.bfloat16

# even oy first, then odd: makes H-pass outputs row-pair interleavable
_OY_PERM = np.concatenate([np.arange(0, OUT, 2), np.arange(1, OUT, 2)])


# ---------------------------------------------------------------- host prep

def _axis_tab(ca, cb, cn):
    i = np.arange(OUT, dtype=np.float32)
    s = np.clip((i + np.float32(0.5)) * cn / np.float32(OUT) - np.float32(0.5),
                np.float32(0.0), cn - np.float32(1.0))
    i0 = np.floor(s).astype(np.int32)
    w = s - i0.astype(np.float32)
    i1 = np.minimum(i0 + 1, cb - ca - 1)
    wstart = min(int(ca), IMG - WIN)
    tab = np.zeros((WIN, OUT), dtype=np.float32)
    np.add.at(tab, (int(ca) - wstart + i0, np.arange(OUT)), (np.float32(1.0) - w))
    np.add.at(tab, (int(ca) - wstart + i1, np.arange(OUT)), w)
    return wstart, tab


def _prep(box_all):
    """-> tabs [B, 120, 898] bf16: RyT even/odd rows + RxT halves at
    [0:896], HWC pair-gather offset (i32 as 2xbf16) at [896:898]."""
    B = box_all.shape[0]
    tabs = np.zeros((B, 120, 4 * OUT + 2), dtype=BF)
    p = np.arange(120, dtype=np.int64)
    for s in range(B):
        b = box_all[s].astype(np.float32) * np.float32(IMG)
        xa = np.int32(np.trunc(b[0] - np.float32(0.5) * b[2]))
        ya = np.int32(np.trunc(b[1] - np.float32(0.5) * b[3]))
        xb = np.int32(np.trunc(b[0] + np.float32(0.5) * b[2]))
        yb = np.int32(np.trunc(b[1] + np.float32(0.5) * b[3]))
        wy0, ryt = _axis_tab(ya, yb, np.float32(yb - ya))
        wx0, rxt = _axis_tab(xa, xb, np.float32(xb - xa))
        ryp = ryt[:, _OY_PERM]
        tabs[s, :, 0:OUT] = ryp[0::2].astype(BF)
        tabs[s, :, OUT:2 * OUT] = ryp[1::2].astype(BF)
        tabs[s, :, 2 * OUT:3 * OUT] = rxt[0:120].astype(BF)
        tabs[s, :, 3 * OUT:4 * OUT] = rxt[120:240].astype(BF)
        # gather offsets ride in the same DMA (sample index is shard-local)
        roff = (((s % NSAMP) * IMG + wy0 + 2 * p) * IMG
                + wx0).astype(np.int32) * np.int32(3)
        tabs[s, :, 4 * OUT:4 * OUT + 2] = (
            np.ascontiguousarray(roff).view(BF).reshape(120, 2))
    return tabs


# ------------------------------------------------- walrus wait-limit fixups

class _SplitDrainTileContext(tile.TileContext):
    """The walrus build here rejects instructions carrying several sync
    waits; re-emit the kernel-tail drain's waits as single-wait NoOps."""

    def _drain_and_barrier(self, tick_clock, wait_clock):
        nc = self.nc
        probe = nc.sync.nop(nofuse=True, hint="drain_wait_probe")
        wait_clock.add_sem_waits(
            probe.ins, ScopedClock({None: tick_clock.global_clock}))
        si = probe.ins.sync_info
        waits = list(si.on_wait) if si is not None else []
        if si is not None:
            si.on_wait = waits[:1]
        for w in waits[1:]:
            n = nc.sync.nop(nofuse=True, hint="drain_wait_split")
            n.ins.sync_info = mybir.SyncInfo(on_wait=[w], on_update=[])
        nc.sync.drain()

        nc.all_engine_barrier()
        assert self.sems is not None
        popped = nc._tile_sem_poison_stack.pop()
        assert popped is self._sem_poison
        nc.clear_and_free_semaphores(list(self.sems.allocated().values()))
        nc.all_engine_barrier()


def _split_sync_waits(nc, max_waits=1):
    ctr = 0
    for fn in nc.m.functions:
        for blk in fn.blocks:
            out = []
            for inst in blk.instructions:
                si = getattr(inst, "sync_info", None)
                waits = list(si.on_wait) if si is not None and si.on_wait else []
                if len(waits) > max_waits:
                    for w in waits[:-max_waits]:
                        ctr += 1
                        out.append(mybir.InstNoOp(
                            name=f"wsplit_{ctr}",
                            engine=inst.engine,
                            ins=[], outs=[],
                            sync_info=mybir.SyncInfo(on_wait=[w], on_update=[])))
                    si.on_wait = waits[-max_waits:]
                out.append(inst)
            blk.instructions = out


# ------------------------------------------------------------ device kernel

def build_kernel(nsamp=NSAMP, n_cores=N_CORES):
    nc = bass.Bass("TRN2", target_bir_lowering=False, debug=False,
                   num_devices=n_cores, dynamic_dma_scratch_size=65536,
                   enable_asserts=False)
    img = nc.dram_tensor("img", [nsamp, IMG, IMG, 3], BF16,
                         kind="ExternalInput")
    tabs = nc.dram_tensor("tabs", [nsamp, 120, 4 * OUT + 2], BF16,
                          kind="ExternalInput")
    out = nc.dram_tensor("out", [nsamp, 3, OUT, OUT], BF16,
                         kind="ExternalOutput")

    N = nsamp * IMG * IMG * 3
    _f = img.ap().rearrange("a b c d -> (a b c d)")
    img_flat = bass.AP(_f.tensor, _f.offset, [[1, N], [1, 1]])

    with _SplitDrainTileContext(nc) as tc, ExitStack() as ctx:
        tabp = ctx.enter_context(tc.tile_pool(name="tabp", bufs=6))
        cwp = ctx.enter_context(tc.tile_pool(name="cwp", bufs=6))
        midp = ctx.enter_context(tc.tile_pool(name="midp", bufs=5))
        outp = ctx.enter_context(tc.tile_pool(name="outp", bufs=4))
        midps = ctx.enter_context(tc.tile_pool(name="midps", bufs=4, space="PSUM"))
        outps = ctx.enter_context(tc.tile_pool(name="outps", bufs=4, space="PSUM"))

        tabs_sb = {}
        out_sb = {}
        mid_sb = {}
        cw_sb = {}

        def emit_front(i):
            """Per-sample gather, V-pass matmuls, PSUM->SBUF cast."""
            s, c = divmod(i, 3)
            if c == 0:
                t = tabp.tile([120, 4 * OUT + 2], BF16, tag="tab",
                              name=f"tab{s}")
                nc.sync.dma_start(t[:], tabs.ap()[s])
                tabs_sb[s] = t
                out_sb[s] = outp.tile([112, 3 * 2 * OUT], BF16, tag="osb",
                                      name=f"osb{s}")
                w = cwp.tile([120, RUN], BF16, tag="cw", name=f"cw{s}")
                nc.gpsimd.indirect_dma_start(
                    out=w[:], out_offset=None, in_=img_flat,
                    in_offset=bass.IndirectOffsetOnAxis(
                        ap=t[:, 4 * OUT:4 * OUT + 2].bitcast(I32), axis=0))
                cw_sb[s] = w
            # V pass: mid[x, oy'] over HWC row pairs (even-row window at
            # cw[:,0:720), odd at cw[:,1440:2160), channel = stride-3 phase)
            mid_ps = midps.tile([120, 2 * OUT], F32, tag="mps", name=f"mps{i}")
            t = tabs_sb[s]
            for xc in range(2):
                for par in range(2):
                    o = par * 1440 + xc * 360 + c
                    nc.tensor.matmul(
                        mid_ps[:, xc * OUT:(xc + 1) * OUT],
                        lhsT=cw_sb[s][:, o:o + 358:3],
                        rhs=t[:, par * OUT:(par + 1) * OUT],
                        start=(par == 0), stop=(par == 1))
            m = midp.tile([120, 2 * OUT], BF16, tag="mid", name=f"mid{i}")
            nc.vector.tensor_copy(m[:], mid_ps[:])
            mid_sb[i] = m

        def emit_back(i):
            """H-pass matmuls for channel i, copy-out, sample write."""
            s, c = divmod(i, 3)
            t = tabs_sb[s]
            m = mid_sb.pop(i)
            out_ps = outps.tile([112, 2 * OUT], F32, tag="ops", name=f"ops{i}")
            for par in range(2):
                for xc in range(2):
                    nc.tensor.matmul(
                        out_ps[:, par * OUT:(par + 1) * OUT],
                        lhsT=m[:, xc * OUT + par * 112:
                               xc * OUT + par * 112 + 112],
                        rhs=t[:, (2 + xc) * OUT:(3 + xc) * OUT],
                        start=(xc == 0), stop=(xc == 1))
            nc.scalar.copy(out=out_sb[s][:, c * 448:(c + 1) * 448],
                           in_=out_ps[:])
            if c == 2:
                # one write per sample: [112 partitions, (c, oy-parity, ox)]
                dst = bass.AP(out.ap().tensor, s * 3 * OUT * OUT,
                              [[2 * OUT, 112], [OUT * OUT, 3], [OUT, 2],
                               [1, OUT]])
                nc.sync.dma_start(dst, out_sb[s][:])

        nch = nsamp * 3
        for i in range(nch):
            emit_front(i)
            if i > 1:
                emit_back(i - 2)
        emit_back(nch - 2)
        emit_back(nch - 1)
    _split_sync_waits(nc)
    return nc


_NC_CACHE = {}


def _run(img, box, trace=False, trace_kwargs=None):
    key = (NSAMP, N_CORES)
    if key not in _NC_CACHE:
        _NC_CACHE[key] = build_kernel(*key)
    nc = _NC_CACHE[key]
    tabs = _prep(np.asarray(box, dtype=np.float32))
    img = np.asarray(img, dtype=np.float32).astype(BF)
    in_maps = []
    for cid in range(N_CORES):
        lo = cid * NSAMP
        hwc = np.ascontiguousarray(
            np.transpose(img[lo:lo + NSAMP], (0, 2, 3, 1)))
        in_maps.append({
            "img": hwc,
            "tabs": np.ascontiguousarray(tabs[lo:lo + NSAMP]),
        })
    res = run_bass_kernel_spmd(nc, in_maps, list(range(N_CORES)), trace=trace,
                               **(trace_kwargs or {}))
    full = np.concatenate([np.asarray(res.results[i]["out"])
                           for i in range(N_CORES)], axis=0)
    return full.reshape(BATCH, 3 * OUT * OUT).astype(np.float32), res


def kernel(img, box):
    out, _ = _run(img, box, trace=False)
    return out
